# revision 10
# baseline (speedup 1.0000x reference)
"""Trainium2 Bass kernel for nn_BlockUpdateBlock (e3nn 'uvu' tensor product).

out = (block + TP(block, others, weights), others)

Strategy ("natural-M"): for each output slot (k,u),
    out_k[n,u] = sum_i x_i[n,u] * M_(k,i)[n,u]
where M is LINEAR in `others`, so M = y^T.T @ W is ONE PE matmul per 128-row
chunk (output in natural [n, 1504] layout). W [256,1504] folds the per-path
weights, Wigner-3j couplings, e3nn normalization, and the +1 residual (via a
constant ones-row baked into the host-pre-transposed `others_t` input, so no
on-device transposes are needed at all).

float32r (~13-bit mantissa) matmuls run at full PE rate; fp32 accuracy is
restored by a 3-term compensated split computed on device:
    M = Wh.yh + Wh.yl + Wl.yh   (hi = f32r round, lo = f32r(exact - hi))
giving rel err ~2.4e-7 vs the fp32 reference (measured on HW).

DVE does 3 broadcast-strided product ops + 6 tree-add ops per chunk; ACT does
the f32r rounding copies; chunks are software-pipelined (y-prep 3 ahead) so
the PE never idles. Pure data-parallel over 8 NeuronCores (12500 rows each,
padded to 12544 = 98 chunks of 128).

Measured: correctness rel err 2.42e-7; cost-model timeline ~406 us/core.
"""
import numpy as np
from contextlib import ExitStack
from math import factorial, sqrt

# ---------------------------------------------------------------- w3j math
def _w3j_su2(j1, j2, j3, m1, m2, m3):
    if m1 + m2 + m3 != 0 or not (abs(j1 - j2) <= j3 <= j1 + j2):
        return 0.0
    f = factorial
    pre = ((-1) ** (j1 - j2 - m3)) * sqrt(
        f(j1 + j2 - j3) * f(j1 - j2 + j3) * f(-j1 + j2 + j3) / f(j1 + j2 + j3 + 1)
        * f(j1 + m1) * f(j1 - m1) * f(j2 + m2) * f(j2 - m2) * f(j3 + m3) * f(j3 - m3))
    kmin = max(0, j2 - j3 - m1, j1 - j3 + m2)
    kmax = min(j1 + j2 - j3, j1 - m1, j2 + m2)
    s = 0.0
    for k in range(kmin, kmax + 1):
        s += ((-1) ** k) / (f(k) * f(j1 + j2 - j3 - k) * f(j1 - m1 - k)
                            * f(j2 + m2 - k) * f(j3 - j2 + m1 + k) * f(j3 - j1 - m2 + k))
    return pre * s


def _real_basis(l):
    U = np.zeros((2 * l + 1, 2 * l + 1), dtype=np.complex128)
    for m in range(-l, l + 1):
        r = m + l
        if m == 0:
            U[r, l] = 1.0
        elif m > 0:
            U[r, -m + l] = 1.0 / np.sqrt(2.0)
            U[r, m + l] = ((-1.0) ** m) / np.sqrt(2.0)
        else:
            a = -m
            U[r, -a + l] = 1j / np.sqrt(2.0)
            U[r, a + l] = -((-1.0) ** a) * 1j / np.sqrt(2.0)
    return U


def _real_wigner_3j(l1, l2, l3):
    Cc = np.zeros((2 * l1 + 1, 2 * l2 + 1, 2 * l3 + 1), dtype=np.complex128)
    for a in range(2 * l1 + 1):
        for b in range(2 * l2 + 1):
            for c in range(2 * l3 + 1):
                Cc[a, b, c] = _w3j_su2(l1, l2, l3, a - l1, b - l2, c - l3)
    T = np.einsum('ia,jb,kc,abc->ijk', _real_basis(l1), _real_basis(l2), _real_basis(l3), Cc)
    re, im = np.real(T), np.imag(T)
    C = re if np.abs(re).max() >= np.abs(im).max() else im
    return (C / np.linalg.norm(C)).astype(np.float64)


# ------------------------------------------------------------ W construction
# M layout (1504 cols): [M0: u(128) | M1: (k3,i3,u64)=576 | M2: (k5,i5,u32)=800]
# y^T rows = NATURAL others features (plain transposes of contiguous slices):
#   A[0:128] = others feats 0:128;  B[0:112] = feats 128:240;  B[112] = ones;
#   B[113:128] = zero pad.  Global row index: feat f -> row f; ones -> row 240.
N_M = 1504

def _build_W(weights: np.ndarray):
    C2 = _real_wigner_3j(1, 1, 1)
    C3 = _real_wigner_3j(1, 2, 1)
    C5 = _real_wigner_3j(2, 1, 2)
    C6 = _real_wigner_3j(2, 2, 2)
    D1 = _real_wigner_3j(1, 0, 1)[0, 0, 0]
    D4 = _real_wigner_3j(2, 0, 2)[0, 0, 0]
    C0 = _real_wigner_3j(0, 0, 0)[0, 0, 0]
    COEF0 = sqrt(1.0 / 64.0)
    COEF1 = sqrt(3.0 / 112.0)
    COEF2 = sqrt(5.0 / 112.0)

    w = weights.astype(np.float64)
    o = [0, 8192, 12288, 14336, 15360, 17408, 18432, 18944]
    w0 = w[o[0]:o[1]].reshape(128, 64)
    w1 = w[o[1]:o[2]].reshape(64, 64)
    w2 = w[o[2]:o[3]].reshape(64, 32)
    w3 = w[o[3]:o[4]].reshape(64, 16)
    w4 = w[o[4]:o[5]].reshape(32, 64)
    w5 = w[o[5]:o[6]].reshape(32, 32)
    w6 = w[o[6]:o[7]].reshape(32, 16)

    WG = np.zeros((241, N_M), dtype=np.float64)   # rows: feats 0:240 + ones(240)
    ONES = 240

    def y1row(v, j):
        return 64 + 3 * v + j

    def y2row(v, j):
        return 160 + 5 * v + j

    # M0 cols: u
    mc0 = np.arange(128)
    WG[0:64, mc0] += (COEF0 * C0) * w0.T        # [64v, 128u]
    WG[ONES, mc0] += 1.0
    # M1 cols: 128 + k*192 + i*64 + u
    for k in range(3):
        for i in range(3):
            cols = 128 + k * 192 + i * 64 + np.arange(64)
            if i == k:
                WG[0:64, cols] += (COEF1 * D1) * w1.T
                WG[ONES, cols] += 1.0
            for j in range(3):
                c = COEF1 * C2[i, j, k]
                if abs(c) > 1e-12:
                    for v in range(32):
                        WG[y1row(v, j), cols] += c * w2[:, v]
            for j in range(5):
                c = COEF1 * C3[i, j, k]
                if abs(c) > 1e-12:
                    for v in range(16):
                        WG[y2row(v, j), cols] += c * w3[:, v]
    # M2 cols: 704 + k*160 + i*32 + u
    for k in range(5):
        for i in range(5):
            cols = 704 + k * 160 + i * 32 + np.arange(32)
            if i == k:
                WG[0:64, cols] += (COEF2 * D4) * w4.T
                WG[ONES, cols] += 1.0
            for j in range(3):
                c = COEF2 * C5[i, j, k]
                if abs(c) > 1e-12:
                    for v in range(32):
                        WG[y1row(v, j), cols] += c * w5[:, v]
            for j in range(5):
                c = COEF2 * C6[i, j, k]
                if abs(c) > 1e-12:
                    for v in range(16):
                        WG[y2row(v, j), cols] += c * w6[:, v]
    WA = WG[0:128]
    WB = np.zeros((128, N_M), dtype=np.float64)
    WB[0:112] = WG[128:240]
    WB[112] = WG[240]
    return WA.astype(np.float32), WB.astype(np.float32)


# ------------------------------------------------------------- Bass program
N_CORES = 8
ROWS_PER_CORE = 12500
ROWS_PAD = 12544            # 98 chunks of 128
N_CHUNKS = 98
SUP = 7                     # chunks per supertile
N_SUP = N_CHUNKS // SUP     # 14


def _build_nc():
    import concourse.bass as bass
    import concourse.tile as tile
    from concourse import bacc, mybir

    F32 = mybir.dt.float32
    F32R = mybir.dt.float32r
    AF = mybir.ActivationFunctionType

    nc = bacc.Bacc("TRN2", target_bir_lowering=False, debug=False)

    blk_d = nc.dram_tensor("block", [ROWS_PAD, 480], F32, kind="ExternalInput").ap()
    oth_d = nc.dram_tensor("others_t", [256, ROWS_PAD], F32, kind="ExternalInput").ap()
    wa_d = nc.dram_tensor("wa", [128, N_M], F32, kind="ExternalInput").ap()
    wb_d = nc.dram_tensor("wb", [128, N_M], F32, kind="ExternalInput").ap()
    id_d = nc.dram_tensor("ident", [128, 128], F32, kind="ExternalInput").ap()
    out_d = nc.dram_tensor("out", [ROWS_PAD, 480], F32, kind="ExternalOutput").ap()

    with tile.TileContext(nc) as tc, ExitStack() as ctx:
        cpool = ctx.enter_context(tc.tile_pool(name="const", bufs=1))
        stg_pool = ctx.enter_context(tc.tile_pool(name="stg", bufs=1))
        wa_hi = cpool.tile([128, N_M], F32R)
        wa_lo = cpool.tile([128, N_M], F32R)
        wb_hi = cpool.tile([128, N_M], F32R)
        wb_lo = cpool.tile([128, N_M], F32R)
        wstg = stg_pool.tile([128, N_M], F32, tag="wstg")
        nc.sync.dma_start(wstg[:], wa_d[:])
        nc.vector.tensor_copy(wa_hi[:], wstg[:])           # rounds f32 -> f32r
        nc.vector.tensor_sub(wa_lo[:], wstg[:], wa_hi[:])  # residual, rounded
        wstg2 = stg_pool.tile([128, N_M], F32, tag="wstg2")
        nc.sync.dma_start(wstg2[:], wb_d[:])
        nc.vector.tensor_copy(wb_hi[:], wstg2[:])
        nc.vector.tensor_sub(wb_lo[:], wstg2[:], wb_hi[:])
        ident = cpool.tile([128, 128], F32)
        nc.sync.dma_start(ident[:], id_d[:])

        io = ctx.enter_context(tc.tile_pool(name="io", bufs=3))
        mp = ctx.enter_context(tc.tile_pool(name="mp", bufs=2, space="PSUM"))
        yts = ctx.enter_context(tc.tile_pool(name="yts", bufs=5))
        tmp = ctx.enter_context(tc.tile_pool(name="tmp", bufs=2))

        def prep_y(oth_tile, c):
            """slice pre-transposed others -> merged y^T hi/lo [128,256] f32r.
            oth_tile: [128, 2, SUP*128] staging (halves: A = yT rows 0:128,
            B = rows 128:256 incl baked ones row at B[112], zeros after)."""
            yv = oth_tile[:, :, c * 128:(c + 1) * 128]   # [128, 2, 128]
            hi = yts.tile([128, 256], F32R, tag="hi")
            lo = yts.tile([128, 256], F32R, tag="lo")
            hv = hi[:].rearrange("p (h n) -> p h n", h=2)
            lv = lo[:].rearrange("p (h n) -> p h n", h=2)
            nc.scalar.copy(hv, yv)                 # rounds f32 -> f32r
            nc.vector.tensor_sub(lv, yv, hv)       # baked 1.0/0.0 -> exact 0
            return hi, lo

        def matmuls(hi, lo):
            m = mp.tile([128, N_M], F32)
            for (c0, c1) in [(0, 512), (512, 1024), (1024, N_M)]:
                nc.tensor.matmul(m[:, c0:c1], lhsT=hi[:, 0:128],
                                 rhs=wa_hi[:, c0:c1], start=True, stop=False)
                nc.tensor.matmul(m[:, c0:c1], lhsT=hi[:, 128:256],
                                 rhs=wb_hi[:, c0:c1], start=False, stop=False)
                nc.tensor.matmul(m[:, c0:c1], lhsT=lo[:, 0:128],
                                 rhs=wa_hi[:, c0:c1], start=False, stop=False)
                nc.tensor.matmul(m[:, c0:c1], lhsT=lo[:, 128:256],
                                 rhs=wb_hi[:, c0:c1], start=False, stop=False)
                nc.tensor.matmul(m[:, c0:c1], lhsT=hi[:, 0:128],
                                 rhs=wa_lo[:, c0:c1], start=False, stop=False)
                nc.tensor.matmul(m[:, c0:c1], lhsT=hi[:, 128:256],
                                 rhs=wb_lo[:, c0:c1], start=False, stop=True)
            return m

        def products_adds(blk_tile, outt_tile, c, m):
            x = blk_tile[:, c, :]
            out_c = outt_tile[:, c, :]
            nc.vector.tensor_mul(out_c[:, 0:128], x[:, 0:128], m[:, 0:128])

            tmp1 = tmp.tile([128, 576], F32, tag="t1")
            x1b = (x[:, 128:320].rearrange("p (u i) -> p i u", i=3)
                   .unsqueeze(1).broadcast_to([128, 3, 3, 64]))
            nc.vector.tensor_mul(
                tmp1[:].rearrange("p (k i u) -> p k i u", k=3, i=3),
                x1b, m[:, 128:704].rearrange("p (k i u) -> p k i u", k=3, i=3))

            tmp2 = tmp.tile([128, 800], F32, tag="t2")
            x2b = (x[:, 320:480].rearrange("p (u i) -> p i u", i=5)
                   .unsqueeze(1).broadcast_to([128, 5, 5, 32]))
            nc.vector.tensor_mul(
                tmp2[:].rearrange("p (k i u) -> p k i u", k=5, i=5),
                x2b, m[:, 704:1504].rearrange("p (k i u) -> p k i u", k=5, i=5))

            def t1i(i):
                return tmp1[:].rearrange("p (k i u) -> p k i u", k=3, i=3)[:, :, i, :]
            acc1 = tmp.tile([128, 192], F32, tag="a1")
            a1v = acc1[:].rearrange("p (k u) -> p k u", k=3)
            nc.vector.tensor_add(a1v, t1i(0), t1i(1))
            nc.vector.tensor_add(
                out_c[:, 128:320].rearrange("p (u k) -> p k u", k=3), a1v, t1i(2))

            t2v = tmp2[:].rearrange("p (k i u) -> p k i u", k=5, i=5)
            acc2a = tmp.tile([128, 160], F32, tag="a2a")
            a2av = acc2a[:].rearrange("p (k u) -> p k u", k=5)
            nc.vector.tensor_add(a2av, t2v[:, :, 0, :], t2v[:, :, 1, :])
            acc2b = tmp.tile([128, 160], F32, tag="a2b")
            a2bv = acc2b[:].rearrange("p (k u) -> p k u", k=5)
            nc.vector.tensor_add(a2bv, t2v[:, :, 2, :], t2v[:, :, 3, :])
            acc2c = tmp.tile([128, 160], F32, tag="a2c")
            a2cv = acc2c[:].rearrange("p (k u) -> p k u", k=5)
            nc.vector.tensor_add(a2cv, a2av, a2bv)
            nc.vector.tensor_add(
                out_c[:, 320:480].rearrange("p (u k) -> p k u", k=5),
                a2cv, t2v[:, :, 4, :])

        def load_sup(s):
            r0 = s * SUP * 128
            blk = io.tile([128, SUP, 480], F32, tag="blk")
            nc.sync.dma_start(
                blk[:], blk_d[r0:r0 + SUP * 128, :].rearrange("(c p) f -> p c f", p=128))
            oth = io.tile([128, 2, SUP * 128], F32, tag="oth")
            nc.sync.dma_start(
                oth[:],
                oth_d[:, r0:r0 + SUP * 128].rearrange("(h p) n -> p h n", p=128))
            outt = io.tile([128, SUP, 480], F32, tag="out")
            return blk, oth, outt

        # flat chunk list with 2-ahead y-prep pipelining
        sup_tiles = [None] * N_SUP
        sup_tiles[0] = load_sup(0)
        sup_tiles[1] = load_sup(1)
        AHEAD = 3
        hilo = {}
        for t in range(AHEAD):
            s, c = divmod(t, SUP)
            hilo[t] = prep_y(sup_tiles[s][1], c)
        for t in range(N_CHUNKS):
            s, c = divmod(t, SUP)
            blk, oth, outt = sup_tiles[s]
            m = matmuls(*hilo.pop(t))
            t2 = t + AHEAD
            if t2 < N_CHUNKS:
                s2, c2 = divmod(t2, SUP)
                if sup_tiles[s2] is None:
                    sup_tiles[s2] = load_sup(s2)
                hilo[t2] = prep_y(sup_tiles[s2][1], c2)
            products_adds(blk, outt, c, m)
            if c == SUP - 1:
                r0 = s * SUP * 128
                nc.sync.dma_start(
                    out_d[r0:r0 + SUP * 128, :].rearrange("(c p) f -> p c f", p=128),
                    outt[:])
                # allow next supertile load slot
                if s + 2 < N_SUP and sup_tiles[s + 2] is None:
                    sup_tiles[s + 2] = load_sup(s + 2)

    nc.compile()
    return nc


_NC_CACHE = {}


def kernel(block: np.ndarray, others: np.ndarray, weights: np.ndarray):
    from concourse.bass_utils import run_bass_kernel_spmd

    block = np.ascontiguousarray(block, dtype=np.float32)
    others_in = others
    others = np.ascontiguousarray(others, dtype=np.float32)
    weights = np.ascontiguousarray(weights, dtype=np.float32)
    assert block.shape == (N_CORES * ROWS_PER_CORE, 480)
    assert others.shape == (N_CORES * ROWS_PER_CORE, 240)

    wa, wb = _build_W(weights)
    ident = np.eye(128, dtype=np.float32)

    if "nc" not in _NC_CACHE:
        _NC_CACHE["nc"] = _build_nc()
    nc = _NC_CACHE["nc"]

    in_maps = []
    for i in range(N_CORES):
        r0 = i * ROWS_PER_CORE
        bpad = np.zeros((ROWS_PAD, 480), dtype=np.float32)
        bpad[:ROWS_PER_CORE] = block[r0:r0 + ROWS_PER_CORE]
        otp = np.zeros((256, ROWS_PAD), dtype=np.float32)
        otp[0:240, :ROWS_PER_CORE] = others[r0:r0 + ROWS_PER_CORE].T
        otp[240, :] = 1.0   # ones row, lands at B-half partition 112
        in_maps.append({"block": bpad, "others_t": otp,
                        "wa": wa, "wb": wb, "ident": ident})

    res = run_bass_kernel_spmd(nc, in_maps, core_ids=list(range(N_CORES)))

    out = np.empty((N_CORES * ROWS_PER_CORE, 480), dtype=np.float32)
    for i in range(N_CORES):
        out[i * ROWS_PER_CORE:(i + 1) * ROWS_PER_CORE] = \
            res.results[i]["out"][:ROWS_PER_CORE]
    return (out, others_in)


# revision 11
# speedup vs baseline: 1.0209x; 1.0209x over previous
"""Trainium2 Bass kernel for nn_BlockUpdateBlock (e3nn 'uvu' tensor product).

out = (block + TP(block, others, weights), others)

Strategy ("natural-M"): for each output slot (k,u), out_k = sum_i x_i * M_(k,i)
with M linear in `others` -> one PE matmul M = y^T.T @ W per 128-row chunk,
where W [256,1504] folds the per-path weights, Wigner-3j couplings, e3nn
normalization and the +1 residual (via a constant ones-row in y^T).
DVE does 3 broadcast-strided product ops + 6 add ops per chunk.
Data-parallel over 8 NeuronCores (12500 rows each, padded to 12544).
"""
import numpy as np
from contextlib import ExitStack
from math import factorial, sqrt

# ---------------------------------------------------------------- w3j math
def _w3j_su2(j1, j2, j3, m1, m2, m3):
    if m1 + m2 + m3 != 0 or not (abs(j1 - j2) <= j3 <= j1 + j2):
        return 0.0
    f = factorial
    pre = ((-1) ** (j1 - j2 - m3)) * sqrt(
        f(j1 + j2 - j3) * f(j1 - j2 + j3) * f(-j1 + j2 + j3) / f(j1 + j2 + j3 + 1)
        * f(j1 + m1) * f(j1 - m1) * f(j2 + m2) * f(j2 - m2) * f(j3 + m3) * f(j3 - m3))
    kmin = max(0, j2 - j3 - m1, j1 - j3 + m2)
    kmax = min(j1 + j2 - j3, j1 - m1, j2 + m2)
    s = 0.0
    for k in range(kmin, kmax + 1):
        s += ((-1) ** k) / (f(k) * f(j1 + j2 - j3 - k) * f(j1 - m1 - k)
                            * f(j2 + m2 - k) * f(j3 - j2 + m1 + k) * f(j3 - j1 - m2 + k))
    return pre * s


def _real_basis(l):
    U = np.zeros((2 * l + 1, 2 * l + 1), dtype=np.complex128)
    for m in range(-l, l + 1):
        r = m + l
        if m == 0:
            U[r, l] = 1.0
        elif m > 0:
            U[r, -m + l] = 1.0 / np.sqrt(2.0)
            U[r, m + l] = ((-1.0) ** m) / np.sqrt(2.0)
        else:
            a = -m
            U[r, -a + l] = 1j / np.sqrt(2.0)
            U[r, a + l] = -((-1.0) ** a) * 1j / np.sqrt(2.0)
    return U


def _real_wigner_3j(l1, l2, l3):
    Cc = np.zeros((2 * l1 + 1, 2 * l2 + 1, 2 * l3 + 1), dtype=np.complex128)
    for a in range(2 * l1 + 1):
        for b in range(2 * l2 + 1):
            for c in range(2 * l3 + 1):
                Cc[a, b, c] = _w3j_su2(l1, l2, l3, a - l1, b - l2, c - l3)
    T = np.einsum('ia,jb,kc,abc->ijk', _real_basis(l1), _real_basis(l2), _real_basis(l3), Cc)
    re, im = np.real(T), np.imag(T)
    C = re if np.abs(re).max() >= np.abs(im).max() else im
    return (C / np.linalg.norm(C)).astype(np.float64)


# ------------------------------------------------------------ W construction
# M layout (1504 cols): [M0: u(128) | M1: (k3,i3,u64)=576 | M2: (k5,i5,u32)=800]
# y^T row layout (host-pre-transposed others_t, 256 rows = halves A|B):
#   A[0:64] = feats 0:64 (y0); A[64] = ones; A[65:128] = feats 64:127;
#   B[0] = feat 127; B[1:113] = feats 128:240; B[113:128] = zero pad.
# ones in A => M0's 128 columns have all-zero W_B -> B-side matmul passes
# skip cols 0:128 of the first group.
N_M = 1504

def _row_of(f):
    """global others-feature index -> (half, row)"""
    if f < 64:
        return 0, f
    if f < 127:
        return 0, 65 + (f - 64)
    if f == 127:
        return 1, 0
    return 1, 1 + (f - 128)

def _build_W(weights: np.ndarray):
    C2 = _real_wigner_3j(1, 1, 1)
    C3 = _real_wigner_3j(1, 2, 1)
    C5 = _real_wigner_3j(2, 1, 2)
    C6 = _real_wigner_3j(2, 2, 2)
    D1 = _real_wigner_3j(1, 0, 1)[0, 0, 0]
    D4 = _real_wigner_3j(2, 0, 2)[0, 0, 0]
    C0 = _real_wigner_3j(0, 0, 0)[0, 0, 0]
    COEF0 = sqrt(1.0 / 64.0)
    COEF1 = sqrt(3.0 / 112.0)
    COEF2 = sqrt(5.0 / 112.0)

    w = weights.astype(np.float64)
    o = [0, 8192, 12288, 14336, 15360, 17408, 18432, 18944]
    w0 = w[o[0]:o[1]].reshape(128, 64)
    w1 = w[o[1]:o[2]].reshape(64, 64)
    w2 = w[o[2]:o[3]].reshape(64, 32)
    w3 = w[o[3]:o[4]].reshape(64, 16)
    w4 = w[o[4]:o[5]].reshape(32, 64)
    w5 = w[o[5]:o[6]].reshape(32, 32)
    w6 = w[o[6]:o[7]].reshape(32, 16)

    WG = np.zeros((241, N_M), dtype=np.float64)   # rows: feats 0:240 + ones(240)
    ONES = 240

    def y1row(v, j):
        return 64 + 3 * v + j

    def y2row(v, j):
        return 160 + 5 * v + j

    # M0 cols: u
    mc0 = np.arange(128)
    WG[0:64, mc0] += (COEF0 * C0) * w0.T        # [64v, 128u]
    WG[ONES, mc0] += 1.0
    # M1 cols: 128 + k*192 + i*64 + u
    for k in range(3):
        for i in range(3):
            cols = 128 + k * 192 + i * 64 + np.arange(64)
            if i == k:
                WG[0:64, cols] += (COEF1 * D1) * w1.T
                WG[ONES, cols] += 1.0
            for j in range(3):
                c = COEF1 * C2[i, j, k]
                if abs(c) > 1e-12:
                    for v in range(32):
                        WG[y1row(v, j), cols] += c * w2[:, v]
            for j in range(5):
                c = COEF1 * C3[i, j, k]
                if abs(c) > 1e-12:
                    for v in range(16):
                        WG[y2row(v, j), cols] += c * w3[:, v]
    # M2 cols: 704 + k*160 + i*32 + u
    for k in range(5):
        for i in range(5):
            cols = 704 + k * 160 + i * 32 + np.arange(32)
            if i == k:
                WG[0:64, cols] += (COEF2 * D4) * w4.T
                WG[ONES, cols] += 1.0
            for j in range(3):
                c = COEF2 * C5[i, j, k]
                if abs(c) > 1e-12:
                    for v in range(32):
                        WG[y1row(v, j), cols] += c * w5[:, v]
            for j in range(5):
                c = COEF2 * C6[i, j, k]
                if abs(c) > 1e-12:
                    for v in range(16):
                        WG[y2row(v, j), cols] += c * w6[:, v]
    WA = np.zeros((128, N_M), dtype=np.float64)
    WB = np.zeros((128, N_M), dtype=np.float64)
    for f in range(240):
        h, r = _row_of(f)
        (WA if h == 0 else WB)[r] = WG[f]
    WA[64] = WG[240]   # ones row
    return WA.astype(np.float32), WB.astype(np.float32)


# ------------------------------------------------------------- Bass program
N_CORES = 8
ROWS_PER_CORE = 12500
ROWS_PAD = 12544            # 98 chunks of 128
N_CHUNKS = 98
SUP = 7                     # chunks per supertile
N_SUP = N_CHUNKS // SUP     # 14


def _build_nc():
    import concourse.bass as bass
    import concourse.tile as tile
    from concourse import bacc, mybir

    F32 = mybir.dt.float32
    F32R = mybir.dt.float32r
    AF = mybir.ActivationFunctionType

    nc = bacc.Bacc("TRN2", target_bir_lowering=False, debug=False)

    blk_d = nc.dram_tensor("block", [ROWS_PAD, 480], F32, kind="ExternalInput").ap()
    oth_d = nc.dram_tensor("others_t", [256, ROWS_PAD], F32, kind="ExternalInput").ap()
    wa_d = nc.dram_tensor("wa", [128, N_M], F32, kind="ExternalInput").ap()
    wb_d = nc.dram_tensor("wb", [128, N_M], F32, kind="ExternalInput").ap()
    id_d = nc.dram_tensor("ident", [128, 128], F32, kind="ExternalInput").ap()
    out_d = nc.dram_tensor("out", [ROWS_PAD, 480], F32, kind="ExternalOutput").ap()

    with tile.TileContext(nc) as tc, ExitStack() as ctx:
        cpool = ctx.enter_context(tc.tile_pool(name="const", bufs=1))
        stg_pool = ctx.enter_context(tc.tile_pool(name="stg", bufs=1))
        wa_hi = cpool.tile([128, N_M], F32R)
        wa_lo = cpool.tile([128, N_M], F32R)
        wb_hi = cpool.tile([128, N_M], F32R)
        wb_lo = cpool.tile([128, N_M], F32R)
        wstg = stg_pool.tile([128, N_M], F32, tag="wstg")
        nc.sync.dma_start(wstg[:], wa_d[:])
        nc.vector.tensor_copy(wa_hi[:], wstg[:])           # rounds f32 -> f32r
        nc.vector.tensor_sub(wa_lo[:], wstg[:], wa_hi[:])  # residual, rounded
        wstg2 = stg_pool.tile([128, N_M], F32, tag="wstg2")
        nc.sync.dma_start(wstg2[:], wb_d[:])
        nc.vector.tensor_copy(wb_hi[:], wstg2[:])
        nc.vector.tensor_sub(wb_lo[:], wstg2[:], wb_hi[:])
        ident = cpool.tile([128, 128], F32)
        nc.sync.dma_start(ident[:], id_d[:])

        io = ctx.enter_context(tc.tile_pool(name="io", bufs=3))
        mp = ctx.enter_context(tc.tile_pool(name="mp", bufs=2, space="PSUM"))
        yts = ctx.enter_context(tc.tile_pool(name="yts", bufs=5))
        tmp = ctx.enter_context(tc.tile_pool(name="tmp", bufs=2))

        def prep_y(oth_tile, c):
            """slice pre-transposed others -> merged y^T hi/lo [128,256] f32r.
            oth_tile: [128, 2, SUP*128] staging (halves: A = yT rows 0:128,
            B = rows 128:256 incl baked ones row at B[112], zeros after)."""
            yv = oth_tile[:, :, c * 128:(c + 1) * 128]   # [128, 2, 128]
            hi = yts.tile([128, 256], F32R, tag="hi")
            lo = yts.tile([128, 256], F32R, tag="lo")
            hv = hi[:].rearrange("p (h n) -> p h n", h=2)
            lv = lo[:].rearrange("p (h n) -> p h n", h=2)
            nc.scalar.copy(hv, yv)                 # rounds f32 -> f32r
            nc.vector.tensor_sub(lv, yv, hv)       # baked 1.0/0.0 -> exact 0
            return hi, lo

        def matmuls(hi, lo):
            m = mp.tile([128, N_M], F32)
            for (c0, c1) in [(0, 512), (512, 1024), (1024, N_M)]:
                b0 = max(c0, 128)   # B-half has zero W rows for M0 cols [0:128]
                nc.tensor.matmul(m[:, c0:c1], lhsT=hi[:, 0:128],
                                 rhs=wa_hi[:, c0:c1], start=True, stop=False)
                nc.tensor.matmul(m[:, b0:c1], lhsT=hi[:, 128:256],
                                 rhs=wb_hi[:, b0:c1], start=False, stop=False)
                nc.tensor.matmul(m[:, c0:c1], lhsT=lo[:, 0:128],
                                 rhs=wa_hi[:, c0:c1], start=False, stop=False)
                nc.tensor.matmul(m[:, b0:c1], lhsT=lo[:, 128:256],
                                 rhs=wb_hi[:, b0:c1], start=False, stop=False)
                nc.tensor.matmul(m[:, c0:c1], lhsT=hi[:, 0:128],
                                 rhs=wa_lo[:, c0:c1], start=False, stop=False)
                nc.tensor.matmul(m[:, b0:c1], lhsT=hi[:, 128:256],
                                 rhs=wb_lo[:, b0:c1], start=False, stop=True)
            return m

        def products_adds(blk_tile, outt_tile, c, m):
            x = blk_tile[:, c, :]
            out_c = outt_tile[:, c, :]
            nc.vector.tensor_mul(out_c[:, 0:128], x[:, 0:128], m[:, 0:128])

            tmp1 = tmp.tile([128, 576], F32, tag="t1")
            x1b = (x[:, 128:320].rearrange("p (u i) -> p i u", i=3)
                   .unsqueeze(1).broadcast_to([128, 3, 3, 64]))
            nc.vector.tensor_mul(
                tmp1[:].rearrange("p (k i u) -> p k i u", k=3, i=3),
                x1b, m[:, 128:704].rearrange("p (k i u) -> p k i u", k=3, i=3))

            tmp2 = tmp.tile([128, 800], F32, tag="t2")
            x2b = (x[:, 320:480].rearrange("p (u i) -> p i u", i=5)
                   .unsqueeze(1).broadcast_to([128, 5, 5, 32]))
            nc.vector.tensor_mul(
                tmp2[:].rearrange("p (k i u) -> p k i u", k=5, i=5),
                x2b, m[:, 704:1504].rearrange("p (k i u) -> p k i u", k=5, i=5))

            def t1i(i):
                return tmp1[:].rearrange("p (k i u) -> p k i u", k=3, i=3)[:, :, i, :]
            acc1 = tmp.tile([128, 192], F32, tag="a1")
            a1v = acc1[:].rearrange("p (k u) -> p k u", k=3)
            nc.vector.tensor_add(a1v, t1i(0), t1i(1))
            nc.vector.tensor_add(
                out_c[:, 128:320].rearrange("p (u k) -> p k u", k=3), a1v, t1i(2))

            t2v = tmp2[:].rearrange("p (k i u) -> p k i u", k=5, i=5)
            acc2a = tmp.tile([128, 160], F32, tag="a2a")
            a2av = acc2a[:].rearrange("p (k u) -> p k u", k=5)
            nc.vector.tensor_add(a2av, t2v[:, :, 0, :], t2v[:, :, 1, :])
            acc2b = tmp.tile([128, 160], F32, tag="a2b")
            a2bv = acc2b[:].rearrange("p (k u) -> p k u", k=5)
            nc.vector.tensor_add(a2bv, t2v[:, :, 2, :], t2v[:, :, 3, :])
            acc2c = tmp.tile([128, 160], F32, tag="a2c")
            a2cv = acc2c[:].rearrange("p (k u) -> p k u", k=5)
            nc.vector.tensor_add(a2cv, a2av, a2bv)
            nc.vector.tensor_add(
                out_c[:, 320:480].rearrange("p (u k) -> p k u", k=5),
                a2cv, t2v[:, :, 4, :])

        def load_sup(s):
            r0 = s * SUP * 128
            blk = io.tile([128, SUP, 480], F32, tag="blk")
            nc.sync.dma_start(
                blk[:], blk_d[r0:r0 + SUP * 128, :].rearrange("(c p) f -> p c f", p=128))
            oth = io.tile([128, 2, SUP * 128], F32, tag="oth")
            nc.sync.dma_start(
                oth[:],
                oth_d[:, r0:r0 + SUP * 128].rearrange("(h p) n -> p h n", p=128))
            outt = io.tile([128, SUP, 480], F32, tag="out")
            return blk, oth, outt

        # flat chunk list with 2-ahead y-prep pipelining
        sup_tiles = [None] * N_SUP
        sup_tiles[0] = load_sup(0)
        sup_tiles[1] = load_sup(1)
        AHEAD = 3
        hilo = {}
        for t in range(AHEAD):
            s, c = divmod(t, SUP)
            hilo[t] = prep_y(sup_tiles[s][1], c)
        for t in range(N_CHUNKS):
            s, c = divmod(t, SUP)
            blk, oth, outt = sup_tiles[s]
            m = matmuls(*hilo.pop(t))
            t2 = t + AHEAD
            if t2 < N_CHUNKS:
                s2, c2 = divmod(t2, SUP)
                if sup_tiles[s2] is None:
                    sup_tiles[s2] = load_sup(s2)
                hilo[t2] = prep_y(sup_tiles[s2][1], c2)
            products_adds(blk, outt, c, m)
            if c == SUP - 1:
                r0 = s * SUP * 128
                nc.sync.dma_start(
                    out_d[r0:r0 + SUP * 128, :].rearrange("(c p) f -> p c f", p=128),
                    outt[:])
                # allow next supertile load slot
                if s + 2 < N_SUP and sup_tiles[s + 2] is None:
                    sup_tiles[s + 2] = load_sup(s + 2)

    nc.compile()
    return nc


_NC_CACHE = {}


def kernel(block: np.ndarray, others: np.ndarray, weights: np.ndarray):
    from concourse.bass_utils import run_bass_kernel_spmd

    block = np.ascontiguousarray(block, dtype=np.float32)
    others_in = others
    others = np.ascontiguousarray(others, dtype=np.float32)
    weights = np.ascontiguousarray(weights, dtype=np.float32)
    assert block.shape == (N_CORES * ROWS_PER_CORE, 480)
    assert others.shape == (N_CORES * ROWS_PER_CORE, 240)

    wa, wb = _build_W(weights)
    ident = np.eye(128, dtype=np.float32)

    if "nc" not in _NC_CACHE:
        _NC_CACHE["nc"] = _build_nc()
    nc = _NC_CACHE["nc"]

    in_maps = []
    for i in range(N_CORES):
        r0 = i * ROWS_PER_CORE
        bpad = np.zeros((ROWS_PAD, 480), dtype=np.float32)
        bpad[:ROWS_PER_CORE] = block[r0:r0 + ROWS_PER_CORE]
        oT = others[r0:r0 + ROWS_PER_CORE].T
        otp = np.zeros((256, ROWS_PAD), dtype=np.float32)
        otp[0:64, :ROWS_PER_CORE] = oT[0:64]
        otp[64, :] = 1.0                       # ones row (A-half row 64)
        otp[65:128, :ROWS_PER_CORE] = oT[64:127]
        otp[128, :ROWS_PER_CORE] = oT[127]
        otp[129:241, :ROWS_PER_CORE] = oT[128:240]
        in_maps.append({"block": bpad, "others_t": otp,
                        "wa": wa, "wb": wb, "ident": ident})

    res = run_bass_kernel_spmd(nc, in_maps, core_ids=list(range(N_CORES)))

    out = np.empty((N_CORES * ROWS_PER_CORE, 480), dtype=np.float32)
    for i in range(N_CORES):
        out[i * ROWS_PER_CORE:(i + 1) * ROWS_PER_CORE] = \
            res.results[i]["out"][:ROWS_PER_CORE]
    return (out, others_in)


# revision 12
# speedup vs baseline: 1.0214x; 1.0005x over previous
"""Trainium2 Bass kernel for nn_BlockUpdateBlock (e3nn 'uvu' tensor product).

out = (block + TP(block, others, weights), others)

Strategy ("natural-M"): for each output slot (k,u), out_k = sum_i x_i * M_(k,i)
with M linear in `others` -> one PE matmul M = y^T.T @ W per 128-row chunk,
where W [256,1504] folds the per-path weights, Wigner-3j couplings, e3nn
normalization and the +1 residual (via a constant ones-row in y^T).
DVE does 3 broadcast-strided product ops + 6 add ops per chunk.
Data-parallel over 8 NeuronCores (12500 rows each, padded to 12544).
"""
import numpy as np
from contextlib import ExitStack
from math import factorial, sqrt

# ---------------------------------------------------------------- w3j math
def _w3j_su2(j1, j2, j3, m1, m2, m3):
    if m1 + m2 + m3 != 0 or not (abs(j1 - j2) <= j3 <= j1 + j2):
        return 0.0
    f = factorial
    pre = ((-1) ** (j1 - j2 - m3)) * sqrt(
        f(j1 + j2 - j3) * f(j1 - j2 + j3) * f(-j1 + j2 + j3) / f(j1 + j2 + j3 + 1)
        * f(j1 + m1) * f(j1 - m1) * f(j2 + m2) * f(j2 - m2) * f(j3 + m3) * f(j3 - m3))
    kmin = max(0, j2 - j3 - m1, j1 - j3 + m2)
    kmax = min(j1 + j2 - j3, j1 - m1, j2 + m2)
    s = 0.0
    for k in range(kmin, kmax + 1):
        s += ((-1) ** k) / (f(k) * f(j1 + j2 - j3 - k) * f(j1 - m1 - k)
                            * f(j2 + m2 - k) * f(j3 - j2 + m1 + k) * f(j3 - j1 - m2 + k))
    return pre * s


def _real_basis(l):
    U = np.zeros((2 * l + 1, 2 * l + 1), dtype=np.complex128)
    for m in range(-l, l + 1):
        r = m + l
        if m == 0:
            U[r, l] = 1.0
        elif m > 0:
            U[r, -m + l] = 1.0 / np.sqrt(2.0)
            U[r, m + l] = ((-1.0) ** m) / np.sqrt(2.0)
        else:
            a = -m
            U[r, -a + l] = 1j / np.sqrt(2.0)
            U[r, a + l] = -((-1.0) ** a) * 1j / np.sqrt(2.0)
    return U


def _real_wigner_3j(l1, l2, l3):
    Cc = np.zeros((2 * l1 + 1, 2 * l2 + 1, 2 * l3 + 1), dtype=np.complex128)
    for a in range(2 * l1 + 1):
        for b in range(2 * l2 + 1):
            for c in range(2 * l3 + 1):
                Cc[a, b, c] = _w3j_su2(l1, l2, l3, a - l1, b - l2, c - l3)
    T = np.einsum('ia,jb,kc,abc->ijk', _real_basis(l1), _real_basis(l2), _real_basis(l3), Cc)
    re, im = np.real(T), np.imag(T)
    C = re if np.abs(re).max() >= np.abs(im).max() else im
    return (C / np.linalg.norm(C)).astype(np.float64)


# ------------------------------------------------------------ W construction
# M layout (1504 cols): [M0: u(128) | M1: (k3,i3,u64)=576 | M2: (k5,i5,u32)=800]
# y^T row layout (host-pre-transposed others_t, 256 rows = halves A|B):
#   A[0:64] = feats 0:64 (y0); A[64] = ones; A[65:128] = feats 64:127;
#   B[0] = feat 127; B[1:113] = feats 128:240; B[113:128] = zero pad.
# ones in A => M0's 128 columns have all-zero W_B -> B-side matmul passes
# skip cols 0:128 of the first group.
N_M = 1504

def _row_of(f):
    """global others-feature index -> (half, row)"""
    if f < 64:
        return 0, f
    if f < 127:
        return 0, 65 + (f - 64)
    if f == 127:
        return 1, 0
    return 1, 1 + (f - 128)

def _build_W(weights: np.ndarray):
    C2 = _real_wigner_3j(1, 1, 1)
    C3 = _real_wigner_3j(1, 2, 1)
    C5 = _real_wigner_3j(2, 1, 2)
    C6 = _real_wigner_3j(2, 2, 2)
    D1 = _real_wigner_3j(1, 0, 1)[0, 0, 0]
    D4 = _real_wigner_3j(2, 0, 2)[0, 0, 0]
    C0 = _real_wigner_3j(0, 0, 0)[0, 0, 0]
    COEF0 = sqrt(1.0 / 64.0)
    COEF1 = sqrt(3.0 / 112.0)
    COEF2 = sqrt(5.0 / 112.0)

    w = weights.astype(np.float64)
    o = [0, 8192, 12288, 14336, 15360, 17408, 18432, 18944]
    w0 = w[o[0]:o[1]].reshape(128, 64)
    w1 = w[o[1]:o[2]].reshape(64, 64)
    w2 = w[o[2]:o[3]].reshape(64, 32)
    w3 = w[o[3]:o[4]].reshape(64, 16)
    w4 = w[o[4]:o[5]].reshape(32, 64)
    w5 = w[o[5]:o[6]].reshape(32, 32)
    w6 = w[o[6]:o[7]].reshape(32, 16)

    WG = np.zeros((241, N_M), dtype=np.float64)   # rows: feats 0:240 + ones(240)
    ONES = 240

    def y1row(v, j):
        return 64 + 3 * v + j

    def y2row(v, j):
        return 160 + 5 * v + j

    # M0 cols: u
    mc0 = np.arange(128)
    WG[0:64, mc0] += (COEF0 * C0) * w0.T        # [64v, 128u]
    WG[ONES, mc0] += 1.0
    # M1 cols: 128 + k*192 + i*64 + u
    for k in range(3):
        for i in range(3):
            cols = 128 + k * 192 + i * 64 + np.arange(64)
            if i == k:
                WG[0:64, cols] += (COEF1 * D1) * w1.T
                WG[ONES, cols] += 1.0
            for j in range(3):
                c = COEF1 * C2[i, j, k]
                if abs(c) > 1e-12:
                    for v in range(32):
                        WG[y1row(v, j), cols] += c * w2[:, v]
            for j in range(5):
                c = COEF1 * C3[i, j, k]
                if abs(c) > 1e-12:
                    for v in range(16):
                        WG[y2row(v, j), cols] += c * w3[:, v]
    # M2 cols: 704 + k*160 + i*32 + u
    for k in range(5):
        for i in range(5):
            cols = 704 + k * 160 + i * 32 + np.arange(32)
            if i == k:
                WG[0:64, cols] += (COEF2 * D4) * w4.T
                WG[ONES, cols] += 1.0
            for j in range(3):
                c = COEF2 * C5[i, j, k]
                if abs(c) > 1e-12:
                    for v in range(32):
                        WG[y1row(v, j), cols] += c * w5[:, v]
            for j in range(5):
                c = COEF2 * C6[i, j, k]
                if abs(c) > 1e-12:
                    for v in range(16):
                        WG[y2row(v, j), cols] += c * w6[:, v]
    WA = np.zeros((128, N_M), dtype=np.float64)
    WB = np.zeros((128, N_M), dtype=np.float64)
    for f in range(240):
        h, r = _row_of(f)
        (WA if h == 0 else WB)[r] = WG[f]
    WA[64] = WG[240]   # ones row
    return WA.astype(np.float32), WB.astype(np.float32)


# ------------------------------------------------------------- Bass program
N_CORES = 8
ROWS_PER_CORE = 12500
ROWS_PAD = 12544            # 98 chunks of 128
N_CHUNKS = 98
SUP = 7                     # chunks per supertile
N_SUP = N_CHUNKS // SUP     # 14


def _build_nc():
    import concourse.bass as bass
    import concourse.tile as tile
    from concourse import bacc, mybir

    F32 = mybir.dt.float32
    F32R = mybir.dt.float32r
    AF = mybir.ActivationFunctionType

    nc = bacc.Bacc("TRN2", target_bir_lowering=False, debug=False)

    blk_d = nc.dram_tensor("block", [ROWS_PAD, 480], F32, kind="ExternalInput").ap()
    oth_d = nc.dram_tensor("others_t", [256, ROWS_PAD], F32, kind="ExternalInput").ap()
    wa_d = nc.dram_tensor("wa", [128, N_M], F32, kind="ExternalInput").ap()
    wb_d = nc.dram_tensor("wb", [128, N_M], F32, kind="ExternalInput").ap()
    id_d = nc.dram_tensor("ident", [128, 128], F32, kind="ExternalInput").ap()
    out_d = nc.dram_tensor("out", [ROWS_PAD, 480], F32, kind="ExternalOutput").ap()

    with tile.TileContext(nc) as tc, ExitStack() as ctx:
        cpool = ctx.enter_context(tc.tile_pool(name="const", bufs=1))
        stg_pool = ctx.enter_context(tc.tile_pool(name="stg", bufs=1))
        wa_hi = cpool.tile([128, N_M], F32R)
        wa_lo = cpool.tile([128, N_M], F32R)
        wb_hi = cpool.tile([128, N_M], F32R)
        wb_lo = cpool.tile([128, N_M], F32R)
        wstg = stg_pool.tile([128, N_M], F32, tag="wstg")
        nc.sync.dma_start(wstg[:], wa_d[:])
        nc.vector.tensor_copy(wa_hi[:], wstg[:])           # rounds f32 -> f32r
        nc.vector.tensor_sub(wa_lo[:], wstg[:], wa_hi[:])  # residual, rounded
        wstg2 = stg_pool.tile([128, N_M], F32, tag="wstg2")
        nc.sync.dma_start(wstg2[:], wb_d[:])
        nc.vector.tensor_copy(wb_hi[:], wstg2[:])
        nc.vector.tensor_sub(wb_lo[:], wstg2[:], wb_hi[:])
        ident = cpool.tile([128, 128], F32)
        nc.sync.dma_start(ident[:], id_d[:])

        io = ctx.enter_context(tc.tile_pool(name="io", bufs=3))
        mp = ctx.enter_context(tc.tile_pool(name="mp", bufs=2, space="PSUM"))
        yts = ctx.enter_context(tc.tile_pool(name="yts", bufs=5))
        tmp = ctx.enter_context(tc.tile_pool(name="tmp", bufs=2))

        def prep_y(oth_tile, c):
            """slice pre-transposed others -> merged y^T hi/lo [128,256] f32r.
            oth_tile: [128, 2, SUP*128] staging (halves: A = yT rows 0:128,
            B = rows 128:256 incl baked ones row at B[112], zeros after)."""
            yv = oth_tile[:, :, c * 128:(c + 1) * 128]   # [128, 2, 128]
            hi = yts.tile([128, 256], F32R, tag="hi")
            lo = yts.tile([128, 256], F32R, tag="lo")
            hv = hi[:].rearrange("p (h n) -> p h n", h=2)
            lv = lo[:].rearrange("p (h n) -> p h n", h=2)
            nc.scalar.copy(hv, yv)                 # rounds f32 -> f32r
            nc.vector.tensor_sub(lv, yv, hv)       # baked 1.0/0.0 -> exact 0
            return hi, lo

        def matmuls(hi, lo):
            m = mp.tile([128, N_M], F32)
            # group 0: M0 cols [0:128] are A-only (ones-row in A) -> B skips.
            # group 2: M2 block (2,0) cols [1024:1056] is y2-only (B) -> A
            # skips; B-hi pass goes first there to own start=True clearing.
            for (c0, c1) in [(0, 512), (512, 1024), (1024, N_M)]:
                b0 = max(c0, 128)          # B-side trim (group 0)
                a0 = 1056 if c0 == 1024 else c0   # A-side trim (group 2)
                if c0 == 1024:
                    nc.tensor.matmul(m[:, c0:c1], lhsT=hi[:, 128:256],
                                     rhs=wb_hi[:, c0:c1], start=True, stop=False)
                    nc.tensor.matmul(m[:, a0:c1], lhsT=hi[:, 0:128],
                                     rhs=wa_hi[:, a0:c1], start=False, stop=False)
                else:
                    nc.tensor.matmul(m[:, c0:c1], lhsT=hi[:, 0:128],
                                     rhs=wa_hi[:, c0:c1], start=True, stop=False)
                    nc.tensor.matmul(m[:, b0:c1], lhsT=hi[:, 128:256],
                                     rhs=wb_hi[:, b0:c1], start=False, stop=False)
                nc.tensor.matmul(m[:, a0:c1], lhsT=lo[:, 0:128],
                                 rhs=wa_hi[:, a0:c1], start=False, stop=False)
                nc.tensor.matmul(m[:, b0:c1], lhsT=lo[:, 128:256],
                                 rhs=wb_hi[:, b0:c1], start=False, stop=False)
                nc.tensor.matmul(m[:, a0:c1], lhsT=hi[:, 0:128],
                                 rhs=wa_lo[:, a0:c1], start=False, stop=False)
                nc.tensor.matmul(m[:, b0:c1], lhsT=hi[:, 128:256],
                                 rhs=wb_lo[:, b0:c1], start=False, stop=True)
            return m

        def products_adds(blk_tile, outt_tile, c, m):
            x = blk_tile[:, c, :]
            out_c = outt_tile[:, c, :]
            nc.vector.tensor_mul(out_c[:, 0:128], x[:, 0:128], m[:, 0:128])

            tmp1 = tmp.tile([128, 576], F32, tag="t1")
            x1b = (x[:, 128:320].rearrange("p (u i) -> p i u", i=3)
                   .unsqueeze(1).broadcast_to([128, 3, 3, 64]))
            nc.vector.tensor_mul(
                tmp1[:].rearrange("p (k i u) -> p k i u", k=3, i=3),
                x1b, m[:, 128:704].rearrange("p (k i u) -> p k i u", k=3, i=3))

            tmp2 = tmp.tile([128, 800], F32, tag="t2")
            x2b = (x[:, 320:480].rearrange("p (u i) -> p i u", i=5)
                   .unsqueeze(1).broadcast_to([128, 5, 5, 32]))
            nc.vector.tensor_mul(
                tmp2[:].rearrange("p (k i u) -> p k i u", k=5, i=5),
                x2b, m[:, 704:1504].rearrange("p (k i u) -> p k i u", k=5, i=5))

            def t1i(i):
                return tmp1[:].rearrange("p (k i u) -> p k i u", k=3, i=3)[:, :, i, :]
            acc1 = tmp.tile([128, 192], F32, tag="a1")
            a1v = acc1[:].rearrange("p (k u) -> p k u", k=3)
            nc.vector.tensor_add(a1v, t1i(0), t1i(1))
            nc.vector.tensor_add(
                out_c[:, 128:320].rearrange("p (u k) -> p k u", k=3), a1v, t1i(2))

            t2v = tmp2[:].rearrange("p (k i u) -> p k i u", k=5, i=5)
            acc2a = tmp.tile([128, 160], F32, tag="a2a")
            a2av = acc2a[:].rearrange("p (k u) -> p k u", k=5)
            nc.vector.tensor_add(a2av, t2v[:, :, 0, :], t2v[:, :, 1, :])
            acc2b = tmp.tile([128, 160], F32, tag="a2b")
            a2bv = acc2b[:].rearrange("p (k u) -> p k u", k=5)
            nc.vector.tensor_add(a2bv, t2v[:, :, 2, :], t2v[:, :, 3, :])
            acc2c = tmp.tile([128, 160], F32, tag="a2c")
            a2cv = acc2c[:].rearrange("p (k u) -> p k u", k=5)
            nc.vector.tensor_add(a2cv, a2av, a2bv)
            nc.vector.tensor_add(
                out_c[:, 320:480].rearrange("p (u k) -> p k u", k=5),
                a2cv, t2v[:, :, 4, :])

        def load_sup(s):
            r0 = s * SUP * 128
            blk = io.tile([128, SUP, 480], F32, tag="blk")
            nc.sync.dma_start(
                blk[:], blk_d[r0:r0 + SUP * 128, :].rearrange("(c p) f -> p c f", p=128))
            oth = io.tile([128, 2, SUP * 128], F32, tag="oth")
            nc.sync.dma_start(
                oth[:],
                oth_d[:, r0:r0 + SUP * 128].rearrange("(h p) n -> p h n", p=128))
            outt = io.tile([128, SUP, 480], F32, tag="out")
            return blk, oth, outt

        # flat chunk list with 2-ahead y-prep pipelining
        sup_tiles = [None] * N_SUP
        sup_tiles[0] = load_sup(0)
        sup_tiles[1] = load_sup(1)
        AHEAD = 3
        hilo = {}
        for t in range(AHEAD):
            s, c = divmod(t, SUP)
            hilo[t] = prep_y(sup_tiles[s][1], c)
        for t in range(N_CHUNKS):
            s, c = divmod(t, SUP)
            blk, oth, outt = sup_tiles[s]
            m = matmuls(*hilo.pop(t))
            t2 = t + AHEAD
            if t2 < N_CHUNKS:
                s2, c2 = divmod(t2, SUP)
                if sup_tiles[s2] is None:
                    sup_tiles[s2] = load_sup(s2)
                hilo[t2] = prep_y(sup_tiles[s2][1], c2)
            products_adds(blk, outt, c, m)
            if c == SUP - 1:
                r0 = s * SUP * 128
                nc.sync.dma_start(
                    out_d[r0:r0 + SUP * 128, :].rearrange("(c p) f -> p c f", p=128),
                    outt[:])
                # allow next supertile load slot
                if s + 2 < N_SUP and sup_tiles[s + 2] is None:
                    sup_tiles[s + 2] = load_sup(s + 2)

    nc.compile()
    return nc


_NC_CACHE = {}


def kernel(block: np.ndarray, others: np.ndarray, weights: np.ndarray):
    from concourse.bass_utils import run_bass_kernel_spmd

    block = np.ascontiguousarray(block, dtype=np.float32)
    others_in = others
    others = np.ascontiguousarray(others, dtype=np.float32)
    weights = np.ascontiguousarray(weights, dtype=np.float32)
    assert block.shape == (N_CORES * ROWS_PER_CORE, 480)
    assert others.shape == (N_CORES * ROWS_PER_CORE, 240)

    wa, wb = _build_W(weights)
    ident = np.eye(128, dtype=np.float32)

    if "nc" not in _NC_CACHE:
        _NC_CACHE["nc"] = _build_nc()
    nc = _NC_CACHE["nc"]

    in_maps = []
    for i in range(N_CORES):
        r0 = i * ROWS_PER_CORE
        bpad = np.zeros((ROWS_PAD, 480), dtype=np.float32)
        bpad[:ROWS_PER_CORE] = block[r0:r0 + ROWS_PER_CORE]
        oT = others[r0:r0 + ROWS_PER_CORE].T
        otp = np.zeros((256, ROWS_PAD), dtype=np.float32)
        otp[0:64, :ROWS_PER_CORE] = oT[0:64]
        otp[64, :] = 1.0                       # ones row (A-half row 64)
        otp[65:128, :ROWS_PER_CORE] = oT[64:127]
        otp[128, :ROWS_PER_CORE] = oT[127]
        otp[129:241, :ROWS_PER_CORE] = oT[128:240]
        in_maps.append({"block": bpad, "others_t": otp,
                        "wa": wa, "wb": wb, "ident": ident})

    res = run_bass_kernel_spmd(nc, in_maps, core_ids=list(range(N_CORES)))

    out = np.empty((N_CORES * ROWS_PER_CORE, 480), dtype=np.float32)
    for i in range(N_CORES):
        out[i * ROWS_PER_CORE:(i + 1) * ROWS_PER_CORE] = \
            res.results[i]["out"][:ROWS_PER_CORE]
    return (out, others_in)
